# revision 22
# baseline (speedup 1.0000x reference)
"""Trainium2 Bass kernel for the attention-pooling module (v5).

Reference math (B=32, N=2048, D=512, K=256):
    vIp   = vI @ Wi                                   [B,N,K]
    vQp   = vQ @ Wq + bq                              [B,K]
    ha    = leaky_relu(vIp + vQp[:,None,:], 0.01)     [B,N,K]
    scores= ha @ Wp[:,0] + bp                         [B,N]   (bp cancels in softmax)
    pi    = softmax(scores, -1)                       [B,N]
    out   = einsum("bn,bnk->bk", pi, vIp) + vQp       [B,K]

v5 key identity: with g = vIp + vQp (the prelu pre-activation),
    out = pi @ g            (exactly -- sum(pi) == 1 absorbs the vQp add)
and g is recoverable from the stored activation: g = min(ha, 100*ha).
So the attention tail is a single e-weighted reduction over ha -- which
is already on-chip in [K-part, n-free] layout -- done by a custom DVE op
(min(x,100x)/8 * e, accumulate), with e broadcast across partitions by
GpSimd. vI therefore streams ONCE (fp8 vIT only, 4.2 MiB/core): measured
aggregate HBM DMA bandwidth here is only ~130-190 GB/s, so bytes are the
wall. Other structure:
  - vQp on host; ha stored as 8*prelu(g) so its negative branch
    (0.08*g) stays out of fp8 subnormals; the /8 rides the custom op's
    C1 slot and the scores weights wp absorb the 8.
  - exp reads the [1,512] scores PSUM tiles directly (4 small ACT ops)
    producing the unnormalised e row fp8 + Z via accum -- no DVE casts,
    no SBUF score rows, no transposes.
  - Prelu (== leaky relu) and Exp share one ACT table: zero reloads.
  - Streams striped across the three DMA trigger paths (sync HWDGE,
    ACT HWDGE, gpsimd SWDGE) -- a single queue only sustains ~130 GB/s.
"""

import os
import sys

sys.path.insert(0, "/opt/trn_rl_repo")

import numpy as np
import ml_dtypes
from operator import add as _op_add

from concourse import bass, bacc, tile, mybir
from concourse import dve_ops as _dve_ops
from concourse.dve_spec import C0, C1, Spec, Src0, Src1, Zero, minn
from concourse.dve_spec import lower as _dve_lower
from concourse.dve_uop import DveOpSpec
from concourse.bass_utils import run_bass_kernel_spmd

dt = mybir.dt
F32, BF16, FP8 = dt.float32, dt.bfloat16, dt.float8e4
AF = mybir.ActivationFunctionType
ALU = mybir.AluOpType

B, N, D, K = 32, 2048, 512, 256
NCORES = 8
BLOC = B // NCORES           # 4 batches per core
SUP = 512                    # scores-matmul tile (PSUM-bank limited)
WSUP = 1024                  # vIp supertile / ha ACT width
DC = D // 128                # 4 d chunks
KC = K // 128                # 2 k chunks
NEG = 0.01


def _ref_invlrelu_mul_reduce(in0, in1, s0, s1, imm2):
    x = in0.astype(np.float32)
    b = ((np.minimum(x, x * s0) * s1) * in1).astype(np.float32)
    return b, b.reshape(b.shape[0], -1).sum(axis=-1, keepdims=True)


def _register_invlrelu_op():
    """out = (min(in0, in0*C0) * C1) * in1; accum_out = sum(out).

    With C0=100, C1=1/8 and in0 = 8*prelu(g, 0.01) this recomputes
    g * e inline and row-accumulates it: the whole attention tail."""
    name = "INV_LRELU_MUL_REDUCE_ANT"
    for op in _dve_ops.OPS:
        if op.name == name:
            return op
    spec = Spec(
        body=(minn(Src0, Src0 * C0) * C1) * Src1,
        accum=_op_add,
        accum_init=Zero,
        reference=_ref_invlrelu_mul_reduce,
    )
    row = _dve_ops._CUSTOM_DVE_ROW_BASE + len(_dve_ops.OPS)
    assert row < 0x20
    op = _dve_ops.DveOp(name, spec, subdim=False, uops_sha={})
    # self-pin the lowering sha (the pin guards cross-version drift; we
    # lower and pin in the same process)
    for ver in ("v3", "v4"):
        try:
            r = DveOpSpec(
                name=name, opcode=row, uops=_dve_lower(spec, ver=ver), rd1_en=True
            )
            op.uops_sha[ver] = r.sha(ver)
        except Exception:
            pass
    _dve_ops.OPS.append(op)
    _dve_ops.CUSTOM_DVE_SPECS[name] = spec
    _dve_ops._SUB_OPCODE_FOR_NAME[name] = row
    return op


INVLRELU_OP = _register_invlrelu_op()


def build_nc():
    nc = bacc.Bacc("TRN2", target_bir_lowering=False, debug=False)

    vit_d = nc.dram_tensor("vit", [BLOC, 128, 2, 2, N], FP8, kind="ExternalInput")
    f8pk_d = nc.dram_tensor("f8pk", [128, 1184], FP8, kind="ExternalInput")
    pk32_d = nc.dram_tensor("pk32", [128, 137], F32, kind="ExternalInput")
    out = nc.dram_tensor("out", [BLOC, K], F32, kind="ExternalOutput")

    DEBUG = bool(int(os.environ.get("KERNEL_DEBUG", "0")))
    DBG_B = int(os.environ.get("KERNEL_DEBUG_B", "0"))
    if DEBUG:
        d_erow = nc.dram_tensor("d_erow", [1, N], FP8, kind="ExternalOutput")
        d_z = nc.dram_tensor("d_z", [1, 1], F32, kind="ExternalOutput")
        d_fin = nc.dram_tensor("d_fin", [1, K], F32, kind="ExternalOutput")

    with tile.TileContext(nc) as tc:
        with (
            tc.tile_pool(name="const", bufs=1) as cpool,
            tc.tile_pool(name="stream", bufs=4) as spool,
            tc.tile_pool(name="work", bufs=2) as wpool,
            tc.tile_pool(name="pmm", bufs=3, space=bass.MemorySpace.PSUM) as pmm,
            tc.tile_pool(name="psm", bufs=1, space=bass.MemorySpace.PSUM) as psm,
        ):
            f8pk_sb = cpool.tile([128, 1184], FP8, tag="f8pk")
            pk32_sb = cpool.tile([128, 137], F32, tag="pk32")

            vit_tiles = [
                spool.tile([128, 2, 2, N], FP8, tag="vit", name=f"vit{b}")
                for b in range(BLOC)
            ]

            # The sync-engine HWDGE queue measured ~28 GB/s (its sequencer is
            # saturated with semaphore traffic) while the ACT HWDGE and
            # gpsimd SWDGE queues sustain ~145 GB/s each -- so ALL bulk vit
            # streams go on those two; sync only carries the small weights.
            nc.sync.dma_start(out=f8pk_sb[:], in_=f8pk_d[:])
            nc.sync.dma_start(out=pk32_sb[:], in_=pk32_d[:])

            nc.scalar.dma_start(
                out=vit_tiles[0][:, :, :, 0:1024], in_=vit_d[0][:, :, :, 0:1024]
            )
            nc.scalar.dma_start(out=vit_tiles[1][:], in_=vit_d[1])
            nc.scalar.dma_start(out=vit_tiles[3][:], in_=vit_d[3])

            nc.gpsimd.dma_start(
                out=vit_tiles[0][:, :, :, 1024:N], in_=vit_d[0][:, :, :, 1024:N]
            )
            nc.gpsimd.dma_start(out=vit_tiles[2][:], in_=vit_d[2])

            wi8_sb = f8pk_sb[:, 0:1024].rearrange("p (c i k) -> p c i k", c=2, i=2)
            wp8_sb = f8pk_sb[:, 1024:1056].rearrange("p (i j) -> p i j", i=2)
            ones8_sb = f8pk_sb[:, 1056:1184]       # row 0 = 1.0 (fp8)
            vqpt_sb = pk32_sb[:, 0:8].rearrange("p (c b) -> p c b", c=KC)
            idf_sb = pk32_sb[:, 9:137]

            out_sb = cpool.tile([1, BLOC, K], F32, tag="outb")
            has = [None] * BLOC
            accs = [None] * BLOC
            invzs = [None] * BLOC

            def phase_scores(b):
                vit = vit_tiles[b]
                # ha stays alive until the attention reduce of batch b
                ha = wpool.tile([128, KC, N], FP8, tag="ha")
                has[b] = ha
                e_row = wpool.tile([1, N], FP8, tag="erow", name=f"erow{b}")
                zq = wpool.tile([1, 2], F32, tag="zq")
                acch = wpool.tile([128, KC, 2], F32, tag="acch")
                scr = wpool.tile([128, WSUP], FP8, tag="scr")
                for sp in range(N // WSUP):
                    n0 = sp * WSUP
                    for kc in range(KC):
                        vp = pmm.tile([128, WSUP], F32, tag="vp")
                        for h in range(2):       # matmul out <= 1 PSUM bank
                            for cc in range(2):
                                nc.tensor.matmul(
                                    vp[:, h * SUP : (h + 1) * SUP],
                                    wi8_sb[:, cc, :, kc * 128 : (kc + 1) * 128],
                                    vit[:, cc, :, n0 + h * SUP : n0 + (h + 1) * SUP],
                                    perf_mode=mybir.MatmulPerfMode.DoubleRow,
                                    start=(cc == 0),
                                    stop=(cc == 1),
                                )
                        # ha8 = 8*prelu(g): vp = 16*vIp, scale 0.5 -> 8*vIp,
                        # bias = 8*vQp (host), Prelu is alpha-homogeneous.
                        # Prelu shares the exp_and_others ACT table with Exp:
                        # zero table reloads in steady state.
                        nc.scalar.activation(
                            ha[:, kc, n0 : n0 + WSUP], vp[:], AF.Prelu,
                            bias=vqpt_sb[:, kc, b : b + 1], scale=0.5, alpha=NEG,
                        )
                    # scores for this supertile: two bank-sized halves in one
                    # paired PSUM tile so a single exp reads all 1024
                    scp = psm.tile([1, 2, SUP], F32, tag="small", name=f"scp{b}_{sp}")
                    for h in range(2):
                        nc.tensor.matmul(
                            scp[0:1, h, :], wp8_sb[:, :, 0:1],
                            ha[:, :, n0 + h * SUP : n0 + (h + 1) * SUP],
                            perf_mode=mybir.MatmulPerfMode.DoubleRow,
                            start=True, stop=True,
                        )
                    # e + Z partial straight off PSUM, then broadcast this
                    # half while the next supertile computes
                    nc.scalar.activation(
                        e_row[0:1, n0 : n0 + WSUP],
                        scp[0:1, :, :],
                        AF.Exp, scale=1.0 / 8, accum_out=zq[0:1, sp : sp + 1],
                    )
                    # broadcast e across partitions on the PE: ones^T @ e
                    # lands f32 in PSUM where the DVE reduce reads it
                    ebp = pmm.tile([128, WSUP], F32, tag="vp", name=f"ebp{b}_{sp}")
                    for h in range(2):
                        nc.tensor.matmul(
                            ebp[:, h * SUP : (h + 1) * SUP],
                            ones8_sb[0:1, :],
                            e_row[0:1, n0 + h * SUP : n0 + (h + 1) * SUP],
                            start=True, stop=True,
                        )
                    # att^T partials: fused (min(x,100x)/8)*e pass per kc
                    for kc in range(KC):
                        nc.vector._custom_dve(
                            INVLRELU_OP,
                            out=scr[:],
                            in0=ha[:, kc, n0 : n0 + WSUP],
                            in1=ebp[:],
                            s0=100.0,
                            s1=1.0 / 8,
                            accum_out=acch[:, kc, sp : sp + 1],
                        )
                z = wpool.tile([1, 1], F32, tag="z")
                nc.vector.tensor_tensor(
                    z[:], zq[0:1, 0:1], zq[0:1, 1:2], ALU.add
                )
                invz = wpool.tile([1, 1], F32, tag="invz", name=f"invz{b}")
                invzs[b] = invz
                nc.vector.reciprocal(invz[:], z[:])
                acc = wpool.tile([128, KC], F32, tag="acc", name=f"acc{b}")
                accs[b] = acc
                nc.vector.tensor_tensor(
                    acc[:], acch[:, :, 0], acch[:, :, 1], ALU.add
                )
                if DEBUG and b == DBG_B:
                    nc.sync.dma_start(out=d_erow[:], in_=e_row[:])
                    nc.sync.dma_start(out=d_z[:], in_=z[:])

            def phase_attn(b):
                # transpose att^T back to a [1, K] row and scale by 1/Z
                acc, invz = accs[b], invzs[b]
                outp = psm.tile([1, K], F32, tag="small", name=f"outp{b}")
                for kc in range(KC):
                    nc.tensor.transpose(
                        outp[0:1, kc * 128 : (kc + 1) * 128],
                        acc[:, kc : kc + 1],
                        idf_sb[:],
                    )
                nc.vector.tensor_scalar(
                    out_sb[:, b, :], outp[:], invz[:], None, ALU.mult
                )
                if DEBUG and b == DBG_B:
                    nc.sync.dma_start(out=d_fin[:], in_=out_sb[0:1, b, :])

            # attention-tail PE work (2 tiny transposes) trails by one
            # phase so the DVE reduce has a full scores phase to finish
            for b in range(BLOC + 1):
                if b < BLOC:
                    phase_scores(b)
                if b >= 1:
                    phase_attn(b - 1)

            nc.sync.dma_start(out=out[:, :], in_=out_sb[0:1, :, :])

    nc.compile()
    return nc


_NC = None


def _get_nc():
    global _NC
    if _NC is None:
        _NC = build_nc()
    return _NC


def kernel(vI, vQ, Wi, Wq, bq, Wp, bp, **_unused):
    vI = np.asarray(vI, dtype=np.float32)
    vQ = np.asarray(vQ, dtype=np.float32)
    Wi = np.asarray(Wi, dtype=np.float32)
    Wq = np.asarray(Wq, dtype=np.float32)
    bq = np.asarray(bq, dtype=np.float32)
    Wp = np.asarray(Wp, dtype=np.float32)
    # bp shifts every score equally -> cancels in softmax; ignored.

    f8 = ml_dtypes.float8_e4m3
    vi8 = vI.astype(f8)
    # DoubleRow layout: d = cc*256 + i*128 + p  ->  [B, p, cc, i, N]
    viT = np.ascontiguousarray(
        vi8.transpose(0, 2, 1).reshape(B, 2, 2, 128, N).transpose(0, 3, 1, 2, 4)
    )

    vQp = vQ @ Wq + bq                                           # [B, K] fp32

    wi8_dr = np.ascontiguousarray(
        (Wi * 16.0).reshape(2, 2, 128, K).transpose(2, 0, 1, 3)
    ).reshape(128, 1024)                                          # [128,(cc i K)]
    # ha carries 8x scale; wp stays 1x so scp = 8*scores (exp scale 1/8)
    wp_h = Wp[:, 0].reshape(KC, 128).T                           # [128,KC]
    wp_pad = np.zeros((128, 2, 16), np.float32)
    wp_pad[:, :, 0] = wp_h
    ones_row = np.zeros((128, 128), np.float32)
    ones_row[0, :] = 1.0
    f8pk = np.concatenate(
        [wi8_dr, wp_pad.reshape(128, 32), ones_row], axis=1
    ).astype(f8)                                                  # [128,1184]

    onesc = np.ones((128, 1), np.float32)
    idf = np.eye(128, dtype=np.float32)

    def pk32_for(core):
        vqpc = 8.0 * vQp[core * BLOC : (core + 1) * BLOC]         # [BLOC, K]
        vqpt = vqpc.T.reshape(KC, 128, BLOC).transpose(1, 0, 2)   # [128,KC,BLOC]
        return np.ascontiguousarray(
            np.concatenate([vqpt.reshape(128, KC * BLOC), onesc, idf], axis=1)
        ).astype(np.float32)                                      # [128,137]

    in_maps = []
    for c in range(NCORES):
        in_maps.append(
            {
                "vit": viT[c * BLOC : (c + 1) * BLOC],
                "f8pk": f8pk,
                "pk32": pk32_for(c),
            }
        )

    nc = _get_nc()
    res = run_bass_kernel_spmd(
        nc, in_maps, list(range(NCORES)),
        trace=bool(int(os.environ.get("KERNEL_TRACE", "0"))),
        tmpdir=globals().get("TRACE_TMPDIR"),
    )
    kernel.last_results = res
    return np.concatenate([res.results[c]["out"] for c in range(NCORES)], axis=0)


# revision 23
# speedup vs baseline: 1.3692x; 1.3692x over previous
"""Trainium2 Bass kernel for the attention-pooling module (v5).

Reference math (B=32, N=2048, D=512, K=256):
    vIp   = vI @ Wi                                   [B,N,K]
    vQp   = vQ @ Wq + bq                              [B,K]
    ha    = leaky_relu(vIp + vQp[:,None,:], 0.01)     [B,N,K]
    scores= ha @ Wp[:,0] + bp                         [B,N]   (bp cancels in softmax)
    pi    = softmax(scores, -1)                       [B,N]
    out   = einsum("bn,bnk->bk", pi, vIp) + vQp       [B,K]

v5 key identity: with g = vIp + vQp (the prelu pre-activation),
    out = pi @ g            (exactly -- sum(pi) == 1 absorbs the vQp add)
and g is recoverable from the stored activation: g = min(ha, 100*ha).
So the attention tail is a single e-weighted reduction over ha -- which
is already on-chip in [K-part, n-free] layout -- done by a custom DVE op
(min(x,100x)/8 * e, accumulate), with e broadcast across partitions by
GpSimd. vI therefore streams ONCE (fp8 vIT only, 4.2 MiB/core): measured
aggregate HBM DMA bandwidth here is only ~130-190 GB/s, so bytes are the
wall. Other structure:
  - vQp on host; ha stored as 8*prelu(g) so its negative branch
    (0.08*g) stays out of fp8 subnormals; the /8 rides the custom op's
    C1 slot and the scores weights wp absorb the 8.
  - exp reads the [1,512] scores PSUM tiles directly (4 small ACT ops)
    producing the unnormalised e row fp8 + Z via accum -- no DVE casts,
    no SBUF score rows, no transposes.
  - Prelu (== leaky relu) and Exp share one ACT table: zero reloads.
  - Streams striped across the three DMA trigger paths (sync HWDGE,
    ACT HWDGE, gpsimd SWDGE) -- a single queue only sustains ~130 GB/s.
"""

import os
import sys

sys.path.insert(0, "/opt/trn_rl_repo")

import numpy as np
import ml_dtypes
from operator import add as _op_add

from concourse import bass, bacc, tile, mybir
from concourse import dve_ops as _dve_ops
from concourse.dve_spec import C0, C1, Spec, Src0, Src1, Zero, minn
from concourse.dve_spec import lower as _dve_lower
from concourse.dve_uop import DveOpSpec
from concourse.bass_utils import run_bass_kernel_spmd

dt = mybir.dt
F32, BF16, FP8 = dt.float32, dt.bfloat16, dt.float8e4
AF = mybir.ActivationFunctionType
ALU = mybir.AluOpType

B, N, D, K = 32, 2048, 512, 256
NCORES = 8
BLOC = B // NCORES           # 4 batches per core
SUP = 512                    # scores-matmul tile (PSUM-bank limited)
WSUP = 1024                  # vIp supertile / ha ACT width
DC = D // 128                # 4 d chunks
KC = K // 128                # 2 k chunks
NEG = 0.01


def _ref_invlrelu_mul_reduce(in0, in1, s0, s1, imm2):
    x = in0.astype(np.float32)
    b = ((np.minimum(x, x * s0) * s1) * in1).astype(np.float32)
    return b, b.reshape(b.shape[0], -1).sum(axis=-1, keepdims=True)


def _register_invlrelu_op():
    """out = (min(in0, in0*C0) * C1) * in1; accum_out = sum(out).

    With C0=100, C1=1/8 and in0 = 8*prelu(g, 0.01) this recomputes
    g * e inline and row-accumulates it: the whole attention tail."""
    name = "INV_LRELU_MUL_REDUCE_ANT"
    for op in _dve_ops.OPS:
        if op.name == name:
            return op
    spec = Spec(
        body=(minn(Src0, Src0 * C0) * C1) * Src1,
        accum=_op_add,
        accum_init=Zero,
        reference=_ref_invlrelu_mul_reduce,
    )
    row = _dve_ops._CUSTOM_DVE_ROW_BASE + len(_dve_ops.OPS)
    assert row < 0x20
    op = _dve_ops.DveOp(name, spec, subdim=False, uops_sha={})
    # self-pin the lowering sha (the pin guards cross-version drift; we
    # lower and pin in the same process)
    for ver in ("v3", "v4"):
        try:
            r = DveOpSpec(
                name=name, opcode=row, uops=_dve_lower(spec, ver=ver), rd1_en=True
            )
            op.uops_sha[ver] = r.sha(ver)
        except Exception:
            pass
    _dve_ops.OPS.append(op)
    _dve_ops.CUSTOM_DVE_SPECS[name] = spec
    _dve_ops._SUB_OPCODE_FOR_NAME[name] = row
    return op


INVLRELU_OP = _register_invlrelu_op()


def build_nc():
    nc = bacc.Bacc("TRN2", target_bir_lowering=False, debug=False)

    vit_d = nc.dram_tensor("vit", [BLOC, 128, 2, 2, N], FP8, kind="ExternalInput")
    f8pk_d = nc.dram_tensor("f8pk", [128, 1280], FP8, kind="ExternalInput")
    pk32_d = nc.dram_tensor("pk32", [128, 137], F32, kind="ExternalInput")
    out = nc.dram_tensor("out", [BLOC, K], F32, kind="ExternalOutput")

    DEBUG = bool(int(os.environ.get("KERNEL_DEBUG", "0")))
    DBG_B = int(os.environ.get("KERNEL_DEBUG_B", "0"))
    if DEBUG:
        d_erow = nc.dram_tensor("d_erow", [1, N], FP8, kind="ExternalOutput")
        d_z = nc.dram_tensor("d_z", [1, 1], F32, kind="ExternalOutput")
        d_fin = nc.dram_tensor("d_fin", [1, K], F32, kind="ExternalOutput")

    with tile.TileContext(nc) as tc:
        with (
            tc.tile_pool(name="const", bufs=1) as cpool,
            tc.tile_pool(name="stream", bufs=4) as spool,
            tc.tile_pool(name="work", bufs=2) as wpool,
            tc.tile_pool(name="pmm", bufs=2, space=bass.MemorySpace.PSUM) as pmm,
            tc.tile_pool(name="psm", bufs=1, space=bass.MemorySpace.PSUM) as psm,
        ):
            f8pk_sb = cpool.tile([128, 1280], FP8, tag="f8pk")
            pk32_sb = cpool.tile([128, 137], F32, tag="pk32")

            vit_tiles = [
                spool.tile([128, 2, 2, N], FP8, tag="vit", name=f"vit{b}")
                for b in range(BLOC)
            ]

            # The sync-engine HWDGE queue measured ~28 GB/s (its sequencer is
            # saturated with semaphore traffic) while the ACT HWDGE and
            # gpsimd SWDGE queues sustain ~145 GB/s each -- so ALL bulk vit
            # streams go on those two; sync only carries the small weights.
            nc.sync.dma_start(out=f8pk_sb[:], in_=f8pk_d[:])
            nc.sync.dma_start(out=pk32_sb[:], in_=pk32_d[:])

            nc.scalar.dma_start(
                out=vit_tiles[0][:, :, :, 0:1024], in_=vit_d[0][:, :, :, 0:1024]
            )
            nc.scalar.dma_start(out=vit_tiles[1][:], in_=vit_d[1])
            nc.scalar.dma_start(out=vit_tiles[3][:], in_=vit_d[3])

            nc.gpsimd.dma_start(
                out=vit_tiles[0][:, :, :, 1024:N], in_=vit_d[0][:, :, :, 1024:N]
            )
            nc.gpsimd.dma_start(out=vit_tiles[2][:], in_=vit_d[2])

            wi8_sb = f8pk_sb[:, 0:1024].rearrange("p (c i k) -> p c i k", c=2, i=2)
            # wp replicated across all 128 lhsT columns: the scores matmul
            # then writes scores to EVERY partition -- it is the broadcast
            wp8r_sb = f8pk_sb[:, 1024:1280].rearrange("p (i j) -> p i j", i=2)
            vqpt_sb = pk32_sb[:, 0:8].rearrange("p (c b) -> p c b", c=KC)
            idf_sb = pk32_sb[:, 9:137]

            out_sb = cpool.tile([1, BLOC, K], F32, tag="outb")
            has = [None] * BLOC
            accs = [None] * BLOC
            invzs = [None] * BLOC

            def phase_scores(b):
                vit = vit_tiles[b]
                # ha stays alive until the attention reduce of batch b
                ha = wpool.tile([128, KC, N], FP8, tag="ha")
                has[b] = ha
                e_b = wpool.tile([128, N], FP8, tag="eb")
                zq = wpool.tile([128, 2], F32, tag="zq")
                acch = wpool.tile([128, KC, 2], F32, tag="acch")
                scr = wpool.tile([128, WSUP], FP8, tag="scr")
                for sp in range(N // WSUP):
                    n0 = sp * WSUP
                    for kc in range(KC):
                        vp = pmm.tile([128, WSUP], F32, tag="vp")
                        for h in range(2):       # matmul out <= 1 PSUM bank
                            for cc in range(2):
                                nc.tensor.matmul(
                                    vp[:, h * SUP : (h + 1) * SUP],
                                    wi8_sb[:, cc, :, kc * 128 : (kc + 1) * 128],
                                    vit[:, cc, :, n0 + h * SUP : n0 + (h + 1) * SUP],
                                    perf_mode=mybir.MatmulPerfMode.DoubleRow,
                                    start=(cc == 0),
                                    stop=(cc == 1),
                                )
                        # ha8 = 8*prelu(g): vp = 16*vIp, scale 0.5 -> 8*vIp,
                        # bias = 8*vQp (host), Prelu is alpha-homogeneous.
                        # Prelu shares the exp_and_others ACT table with Exp:
                        # zero table reloads in steady state.
                        nc.scalar.activation(
                            ha[:, kc, n0 : n0 + WSUP], vp[:], AF.Prelu,
                            bias=vqpt_sb[:, kc, b : b + 1], scale=0.5, alpha=NEG,
                        )
                    # scores replicated on all partitions: [128, 1024] PSUM
                    scp = psm.tile([128, WSUP], F32, tag="small", name=f"scp{b}_{sp}")
                    for h in range(2):
                        nc.tensor.matmul(
                            scp[:, h * SUP : (h + 1) * SUP], wp8r_sb[:],
                            ha[:, :, n0 + h * SUP : n0 + (h + 1) * SUP],
                            perf_mode=mybir.MatmulPerfMode.DoubleRow,
                            start=True, stop=True,
                        )
                    # one wide partition-parallel exp: e_b written for all
                    # partitions at once, Z replicated via accum
                    nc.scalar.activation(
                        e_b[:, n0 : n0 + WSUP],
                        scp[:],
                        AF.Exp, scale=1.0 / 8, accum_out=zq[:, sp : sp + 1],
                    )
                    # att^T partials: fused (min(x,100x)/8)*e pass per kc
                    for kc in range(KC):
                        nc.vector._custom_dve(
                            INVLRELU_OP,
                            out=scr[:],
                            in0=ha[:, kc, n0 : n0 + WSUP],
                            in1=e_b[:, n0 : n0 + WSUP],
                            s0=100.0,
                            s1=1.0 / 8,
                            accum_out=acch[:, kc, sp : sp + 1],
                        )
                z = wpool.tile([1, 1], F32, tag="z")
                nc.vector.tensor_tensor(
                    z[:], zq[0:1, 0:1], zq[0:1, 1:2], ALU.add
                )
                invz = wpool.tile([1, 1], F32, tag="invz", name=f"invz{b}")
                invzs[b] = invz
                nc.vector.reciprocal(invz[:], z[:])
                acc = wpool.tile([128, KC], F32, tag="acc", name=f"acc{b}")
                accs[b] = acc
                nc.vector.tensor_tensor(
                    acc[:], acch[:, :, 0], acch[:, :, 1], ALU.add
                )
                if DEBUG and b == DBG_B:
                    nc.sync.dma_start(out=d_erow[:], in_=e_b[0:1, :])
                    nc.sync.dma_start(out=d_z[:], in_=z[:])

            def phase_attn(b):
                # transpose att^T back to a [1, K] row and scale by 1/Z
                acc, invz = accs[b], invzs[b]
                outp = psm.tile([1, K], F32, tag="outp", name=f"outp{b}")
                for kc in range(KC):
                    nc.tensor.transpose(
                        outp[0:1, kc * 128 : (kc + 1) * 128],
                        acc[:, kc : kc + 1],
                        idf_sb[:],
                    )
                nc.vector.tensor_scalar(
                    out_sb[:, b, :], outp[:], invz[:], None, ALU.mult
                )
                if DEBUG and b == DBG_B:
                    nc.sync.dma_start(out=d_fin[:], in_=out_sb[0:1, b, :])

            # attention-tail PE work (2 tiny transposes) trails by one
            # phase so the DVE reduce has a full scores phase to finish
            for b in range(BLOC + 1):
                if b < BLOC:
                    phase_scores(b)
                if b >= 1:
                    phase_attn(b - 1)

            nc.sync.dma_start(out=out[:, :], in_=out_sb[0:1, :, :])

    nc.compile()
    return nc


_NC = None


def _get_nc():
    global _NC
    if _NC is None:
        _NC = build_nc()
    return _NC


def kernel(vI, vQ, Wi, Wq, bq, Wp, bp, **_unused):
    vI = np.asarray(vI, dtype=np.float32)
    vQ = np.asarray(vQ, dtype=np.float32)
    Wi = np.asarray(Wi, dtype=np.float32)
    Wq = np.asarray(Wq, dtype=np.float32)
    bq = np.asarray(bq, dtype=np.float32)
    Wp = np.asarray(Wp, dtype=np.float32)
    # bp shifts every score equally -> cancels in softmax; ignored.

    f8 = ml_dtypes.float8_e4m3
    vi8 = vI.astype(f8)
    # DoubleRow layout: d = cc*256 + i*128 + p  ->  [B, p, cc, i, N]
    viT = np.ascontiguousarray(
        vi8.transpose(0, 2, 1).reshape(B, 2, 2, 128, N).transpose(0, 3, 1, 2, 4)
    )

    vQp = vQ @ Wq + bq                                           # [B, K] fp32

    wi8_dr = np.ascontiguousarray(
        (Wi * 16.0).reshape(2, 2, 128, K).transpose(2, 0, 1, 3)
    ).reshape(128, 1024)                                          # [128,(cc i K)]
    # ha carries 8x scale; wp stays 1x so scp = 8*scores (exp scale 1/8)
    wp_h = Wp[:, 0].reshape(KC, 128).T                           # [128,KC]
    wp_rep = np.repeat(wp_h[:, :, None], 128, axis=2)            # [128,2,128]
    f8pk = np.concatenate(
        [wi8_dr, wp_rep.reshape(128, 256)], axis=1
    ).astype(f8)                                                  # [128,1280]

    onesc = np.ones((128, 1), np.float32)
    idf = np.eye(128, dtype=np.float32)

    def pk32_for(core):
        vqpc = 8.0 * vQp[core * BLOC : (core + 1) * BLOC]         # [BLOC, K]
        vqpt = vqpc.T.reshape(KC, 128, BLOC).transpose(1, 0, 2)   # [128,KC,BLOC]
        return np.ascontiguousarray(
            np.concatenate([vqpt.reshape(128, KC * BLOC), onesc, idf], axis=1)
        ).astype(np.float32)                                      # [128,137]

    in_maps = []
    for c in range(NCORES):
        in_maps.append(
            {
                "vit": viT[c * BLOC : (c + 1) * BLOC],
                "f8pk": f8pk,
                "pk32": pk32_for(c),
            }
        )

    nc = _get_nc()
    res = run_bass_kernel_spmd(
        nc, in_maps, list(range(NCORES)),
        trace=bool(int(os.environ.get("KERNEL_TRACE", "0"))),
        tmpdir=globals().get("TRACE_TMPDIR"),
    )
    kernel.last_results = res
    return np.concatenate([res.results[c]["out"] for c in range(NCORES)], axis=0)


# revision 24
# speedup vs baseline: 1.3865x; 1.0126x over previous
"""Trainium2 Bass kernel for the attention-pooling module (v5).

Reference math (B=32, N=2048, D=512, K=256):
    vIp   = vI @ Wi                                   [B,N,K]
    vQp   = vQ @ Wq + bq                              [B,K]
    ha    = leaky_relu(vIp + vQp[:,None,:], 0.01)     [B,N,K]
    scores= ha @ Wp[:,0] + bp                         [B,N]   (bp cancels in softmax)
    pi    = softmax(scores, -1)                       [B,N]
    out   = einsum("bn,bnk->bk", pi, vIp) + vQp       [B,K]

v5 key identity: with g = vIp + vQp (the prelu pre-activation),
    out = pi @ g            (exactly -- sum(pi) == 1 absorbs the vQp add)
and g is recoverable from the stored activation: g = min(ha, 100*ha).
So the attention tail is a single e-weighted reduction over ha -- which
is already on-chip in [K-part, n-free] layout -- done by a custom DVE op
(min(x,100x)/8 * e, accumulate), with e broadcast across partitions by
GpSimd. vI therefore streams ONCE (fp8 vIT only, 4.2 MiB/core): measured
aggregate HBM DMA bandwidth here is only ~130-190 GB/s, so bytes are the
wall. Other structure:
  - vQp on host; ha stored as 8*prelu(g) so its negative branch
    (0.08*g) stays out of fp8 subnormals; the /8 rides the custom op's
    C1 slot and the scores weights wp absorb the 8.
  - exp reads the [1,512] scores PSUM tiles directly (4 small ACT ops)
    producing the unnormalised e row fp8 + Z via accum -- no DVE casts,
    no SBUF score rows, no transposes.
  - Prelu (== leaky relu) and Exp share one ACT table: zero reloads.
  - Streams striped across the three DMA trigger paths (sync HWDGE,
    ACT HWDGE, gpsimd SWDGE) -- a single queue only sustains ~130 GB/s.
"""

import os
import sys

sys.path.insert(0, "/opt/trn_rl_repo")

import numpy as np
import ml_dtypes
from operator import add as _op_add

from concourse import bass, bacc, tile, mybir
from concourse import dve_ops as _dve_ops
from concourse.dve_spec import C0, C1, Spec, Src0, Src1, Zero, minn
from concourse.dve_spec import lower as _dve_lower
from concourse.dve_uop import DveOpSpec
from concourse.bass_utils import run_bass_kernel_spmd

dt = mybir.dt
F32, BF16, FP8 = dt.float32, dt.bfloat16, dt.float8e4
AF = mybir.ActivationFunctionType
ALU = mybir.AluOpType

B, N, D, K = 32, 2048, 512, 256
NCORES = 8
BLOC = B // NCORES           # 4 batches per core
SUP = 512                    # scores-matmul tile (PSUM-bank limited)
WSUP = 1024                  # vIp supertile / ha ACT width
DC = D // 128                # 4 d chunks
KC = K // 128                # 2 k chunks
NEG = 0.01


def _ref_invlrelu_mul_reduce(in0, in1, s0, s1, imm2):
    x = in0.astype(np.float32)
    b = ((np.minimum(x, x * s0) * s1) * in1).astype(np.float32)
    return b, b.reshape(b.shape[0], -1).sum(axis=-1, keepdims=True)


def _register_invlrelu_op():
    """out = (min(in0, in0*C0) * C1) * in1; accum_out = sum(out).

    With C0=100, C1=1/8 and in0 = 8*prelu(g, 0.01) this recomputes
    g * e inline and row-accumulates it: the whole attention tail."""
    name = "INV_LRELU_MUL_REDUCE_ANT"
    for op in _dve_ops.OPS:
        if op.name == name:
            return op
    spec = Spec(
        body=(minn(Src0, Src0 * C0) * C1) * Src1,
        accum=_op_add,
        accum_init=Zero,
        reference=_ref_invlrelu_mul_reduce,
    )
    row = _dve_ops._CUSTOM_DVE_ROW_BASE + len(_dve_ops.OPS)
    assert row < 0x20
    op = _dve_ops.DveOp(name, spec, subdim=False, uops_sha={})
    # self-pin the lowering sha (the pin guards cross-version drift; we
    # lower and pin in the same process)
    for ver in ("v3", "v4"):
        try:
            r = DveOpSpec(
                name=name, opcode=row, uops=_dve_lower(spec, ver=ver), rd1_en=True
            )
            op.uops_sha[ver] = r.sha(ver)
        except Exception:
            pass
    _dve_ops.OPS.append(op)
    _dve_ops.CUSTOM_DVE_SPECS[name] = spec
    _dve_ops._SUB_OPCODE_FOR_NAME[name] = row
    return op


INVLRELU_OP = _register_invlrelu_op()


def build_nc():
    nc = bacc.Bacc("TRN2", target_bir_lowering=False, debug=False)

    vit_d = nc.dram_tensor("vit", [BLOC, 128, 2, 2, N], FP8, kind="ExternalInput")
    f8pk_d = nc.dram_tensor("f8pk", [128, 1280], FP8, kind="ExternalInput")
    pk32_d = nc.dram_tensor("pk32", [128, 137], F32, kind="ExternalInput")
    out = nc.dram_tensor("out", [BLOC, K], F32, kind="ExternalOutput")

    DEBUG = bool(int(os.environ.get("KERNEL_DEBUG", "0")))
    DBG_B = int(os.environ.get("KERNEL_DEBUG_B", "0"))
    if DEBUG:
        d_erow = nc.dram_tensor("d_erow", [1, N], FP8, kind="ExternalOutput")
        d_z = nc.dram_tensor("d_z", [1, 1], F32, kind="ExternalOutput")
        d_fin = nc.dram_tensor("d_fin", [1, K], F32, kind="ExternalOutput")

    with tile.TileContext(nc) as tc:
        with (
            tc.tile_pool(name="const", bufs=1) as cpool,
            tc.tile_pool(name="stream", bufs=4) as spool,
            tc.tile_pool(name="work", bufs=2) as wpool,
            tc.tile_pool(name="pmm", bufs=2, space=bass.MemorySpace.PSUM) as pmm,
            tc.tile_pool(name="psm", bufs=1, space=bass.MemorySpace.PSUM) as psm,
        ):
            f8pk_sb = cpool.tile([128, 1280], FP8, tag="f8pk")
            pk32_sb = cpool.tile([128, 137], F32, tag="pk32")

            vit_tiles = [
                spool.tile([128, 2, 2, N], FP8, tag="vit", name=f"vit{b}")
                for b in range(BLOC)
            ]

            # The sync-engine HWDGE queue measured ~28 GB/s (its sequencer is
            # saturated with semaphore traffic) while the ACT HWDGE and
            # gpsimd SWDGE queues sustain ~145 GB/s each -- so ALL bulk vit
            # streams go on those two; sync only carries the small weights.
            nc.sync.dma_start(out=f8pk_sb[:], in_=f8pk_d[:])
            nc.sync.dma_start(out=pk32_sb[:], in_=pk32_d[:])

            nc.scalar.dma_start(
                out=vit_tiles[0][:, :, :, 0:512], in_=vit_d[0][:, :, :, 0:512]
            )
            nc.scalar.dma_start(
                out=vit_tiles[0][:, :, :, 512:1024], in_=vit_d[0][:, :, :, 512:1024]
            )
            nc.scalar.dma_start(out=vit_tiles[1][:], in_=vit_d[1])
            nc.scalar.dma_start(out=vit_tiles[3][:], in_=vit_d[3])

            nc.gpsimd.dma_start(
                out=vit_tiles[0][:, :, :, 1024:N], in_=vit_d[0][:, :, :, 1024:N]
            )
            nc.gpsimd.dma_start(out=vit_tiles[2][:], in_=vit_d[2])

            wi8_sb = f8pk_sb[:, 0:1024].rearrange("p (c i k) -> p c i k", c=2, i=2)
            # wp replicated across all 128 lhsT columns: the scores matmul
            # then writes scores to EVERY partition -- it is the broadcast
            wp8r_sb = f8pk_sb[:, 1024:1280].rearrange("p (i j) -> p i j", i=2)
            vqpt_sb = pk32_sb[:, 0:8].rearrange("p (c b) -> p c b", c=KC)
            idf_sb = pk32_sb[:, 9:137]

            out_sb = cpool.tile([1, BLOC, K], F32, tag="outb")
            has = [None] * BLOC
            accs = [None] * BLOC
            invzs = [None] * BLOC

            def phase_scores(b):
                vit = vit_tiles[b]
                # ha stays alive until the attention reduce of batch b
                ha = wpool.tile([128, KC, N], FP8, tag="ha")
                has[b] = ha
                e_b = wpool.tile([128, N], FP8, tag="eb")
                zq = wpool.tile([128, 1], F32, tag="zq")
                # all four score quarters accumulate into one 4-bank PSUM
                # tile; a single wide exp then drains it
                scp = psm.tile([128, N], F32, tag="scp", name=f"scp{b}")
                # batch 0 uses two narrow leading supertiles so the first
                # ACT fires as soon as the first quarter of vit0 lands
                widths = (SUP, SUP, WSUP) if b == 0 else (WSUP, WSUP)
                n0 = 0
                for w in widths:
                    for kc in range(KC):
                        vp = pmm.tile([128, w], F32, tag="vp", name=f"vp{b}_{n0}_{kc}")
                        for h in range(w // SUP):
                            for cc in range(2):
                                nc.tensor.matmul(
                                    vp[:, h * SUP : (h + 1) * SUP],
                                    wi8_sb[:, cc, :, kc * 128 : (kc + 1) * 128],
                                    vit[:, cc, :, n0 + h * SUP : n0 + (h + 1) * SUP],
                                    perf_mode=mybir.MatmulPerfMode.DoubleRow,
                                    start=(cc == 0),
                                    stop=(cc == 1),
                                )
                        # ha8 = 8*prelu(g): vp = 16*vIp, scale 0.5 -> 8*vIp,
                        # bias = 8*vQp (host). Prelu shares the ACT table
                        # with Exp: zero reloads.
                        nc.scalar.activation(
                            ha[:, kc, n0 : n0 + w], vp[:], AF.Prelu,
                            bias=vqpt_sb[:, kc, b : b + 1], scale=0.5, alpha=NEG,
                        )
                    for h in range(w // SUP):
                        nc.tensor.matmul(
                            scp[:, n0 + h * SUP : n0 + (h + 1) * SUP], wp8r_sb[:],
                            ha[:, :, n0 + h * SUP : n0 + (h + 1) * SUP],
                            perf_mode=mybir.MatmulPerfMode.DoubleRow,
                            start=True, stop=True,
                        )
                    n0 += w
                # one wide partition-parallel exp: e_b for all partitions,
                # Z replicated into every partition of zq via accum
                nc.scalar.activation(
                    e_b[:], scp[:], AF.Exp, scale=1.0 / 8, accum_out=zq[:],
                )
                invz = wpool.tile([1, 1], F32, tag="invz", name=f"invz{b}")
                invzs[b] = invz
                nc.vector.reciprocal(invz[:], zq[0:1, :])
                # att^T[k] = sum_n e[n] * g[k,n]: one fused pass per kc
                acc = wpool.tile([128, KC], F32, tag="acc", name=f"acc{b}")
                accs[b] = acc
                scr = wpool.tile([128, N], FP8, tag="scr")
                for kc in range(KC):
                    nc.vector._custom_dve(
                        INVLRELU_OP,
                        out=scr[:],
                        in0=ha[:, kc, :],
                        in1=e_b[:],
                        s0=100.0,
                        s1=1.0 / 8,
                        accum_out=acc[:, kc : kc + 1],
                    )
                if DEBUG and b == DBG_B:
                    nc.sync.dma_start(out=d_erow[:], in_=e_b[0:1, :])
                    nc.sync.dma_start(out=d_z[:], in_=zq[0:1, :])

            def phase_attn(b):
                # transpose att^T back to a [1, K] row and scale by 1/Z
                acc, invz = accs[b], invzs[b]
                outp = pmm.tile([1, K], F32, tag="vp", name=f"outp{b}")
                for kc in range(KC):
                    nc.tensor.transpose(
                        outp[0:1, kc * 128 : (kc + 1) * 128],
                        acc[:, kc : kc + 1],
                        idf_sb[:],
                    )
                nc.vector.tensor_scalar(
                    out_sb[:, b, :], outp[:], invz[:], None, ALU.mult
                )
                if DEBUG and b == DBG_B:
                    nc.sync.dma_start(out=d_fin[:], in_=out_sb[0:1, b, :])

            # attention-tail PE work (2 tiny transposes) trails by one
            # phase so the DVE reduce has a full scores phase to finish
            for b in range(BLOC + 1):
                if b < BLOC:
                    phase_scores(b)
                if b >= 1:
                    phase_attn(b - 1)

            nc.sync.dma_start(out=out[:, :], in_=out_sb[0:1, :, :])

    nc.compile()
    return nc


_NC = None


def _get_nc():
    global _NC
    if _NC is None:
        _NC = build_nc()
    return _NC


def kernel(vI, vQ, Wi, Wq, bq, Wp, bp, **_unused):
    vI = np.asarray(vI, dtype=np.float32)
    vQ = np.asarray(vQ, dtype=np.float32)
    Wi = np.asarray(Wi, dtype=np.float32)
    Wq = np.asarray(Wq, dtype=np.float32)
    bq = np.asarray(bq, dtype=np.float32)
    Wp = np.asarray(Wp, dtype=np.float32)
    # bp shifts every score equally -> cancels in softmax; ignored.

    f8 = ml_dtypes.float8_e4m3
    vi8 = vI.astype(f8)
    # DoubleRow layout: d = cc*256 + i*128 + p  ->  [B, p, cc, i, N]
    viT = np.ascontiguousarray(
        vi8.transpose(0, 2, 1).reshape(B, 2, 2, 128, N).transpose(0, 3, 1, 2, 4)
    )

    vQp = vQ @ Wq + bq                                           # [B, K] fp32

    wi8_dr = np.ascontiguousarray(
        (Wi * 16.0).reshape(2, 2, 128, K).transpose(2, 0, 1, 3)
    ).reshape(128, 1024)                                          # [128,(cc i K)]
    # ha carries 8x scale; wp stays 1x so scp = 8*scores (exp scale 1/8)
    wp_h = Wp[:, 0].reshape(KC, 128).T                           # [128,KC]
    wp_rep = np.repeat(wp_h[:, :, None], 128, axis=2)            # [128,2,128]
    f8pk = np.concatenate(
        [wi8_dr, wp_rep.reshape(128, 256)], axis=1
    ).astype(f8)                                                  # [128,1280]

    onesc = np.ones((128, 1), np.float32)
    idf = np.eye(128, dtype=np.float32)

    def pk32_for(core):
        vqpc = 8.0 * vQp[core * BLOC : (core + 1) * BLOC]         # [BLOC, K]
        vqpt = vqpc.T.reshape(KC, 128, BLOC).transpose(1, 0, 2)   # [128,KC,BLOC]
        return np.ascontiguousarray(
            np.concatenate([vqpt.reshape(128, KC * BLOC), onesc, idf], axis=1)
        ).astype(np.float32)                                      # [128,137]

    in_maps = []
    for c in range(NCORES):
        in_maps.append(
            {
                "vit": viT[c * BLOC : (c + 1) * BLOC],
                "f8pk": f8pk,
                "pk32": pk32_for(c),
            }
        )

    nc = _get_nc()
    res = run_bass_kernel_spmd(
        nc, in_maps, list(range(NCORES)),
        trace=bool(int(os.environ.get("KERNEL_TRACE", "0"))),
        tmpdir=globals().get("TRACE_TMPDIR"),
    )
    kernel.last_results = res
    return np.concatenate([res.results[c]["out"] for c in range(NCORES)], axis=0)


# revision 25
# speedup vs baseline: 1.5803x; 1.1397x over previous
"""Trainium2 Bass kernel for the attention-pooling module (v5).

Reference math (B=32, N=2048, D=512, K=256):
    vIp   = vI @ Wi                                   [B,N,K]
    vQp   = vQ @ Wq + bq                              [B,K]
    ha    = leaky_relu(vIp + vQp[:,None,:], 0.01)     [B,N,K]
    scores= ha @ Wp[:,0] + bp                         [B,N]   (bp cancels in softmax)
    pi    = softmax(scores, -1)                       [B,N]
    out   = einsum("bn,bnk->bk", pi, vIp) + vQp       [B,K]

v5 key identity: with g = vIp + vQp (the prelu pre-activation),
    out = pi @ g            (exactly -- sum(pi) == 1 absorbs the vQp add)
and g is recoverable from the stored activation: g = min(ha, 100*ha).
So the attention tail is a single e-weighted reduction over ha -- which
is already on-chip in [K-part, n-free] layout -- done by a custom DVE op
(min(x,100x)/8 * e, accumulate), with e broadcast across partitions by
GpSimd. vI therefore streams ONCE (fp8 vIT only, 4.2 MiB/core): measured
aggregate HBM DMA bandwidth here is only ~130-190 GB/s, so bytes are the
wall. Other structure:
  - vQp on host; ha stored as 8*prelu(g) so its negative branch
    (0.08*g) stays out of fp8 subnormals; the /8 rides the custom op's
    C1 slot and the scores weights wp absorb the 8.
  - exp reads the [1,512] scores PSUM tiles directly (4 small ACT ops)
    producing the unnormalised e row fp8 + Z via accum -- no DVE casts,
    no SBUF score rows, no transposes.
  - Prelu (== leaky relu) and Exp share one ACT table: zero reloads.
  - Streams striped across the three DMA trigger paths (sync HWDGE,
    ACT HWDGE, gpsimd SWDGE) -- a single queue only sustains ~130 GB/s.
"""

import os
import sys

sys.path.insert(0, "/opt/trn_rl_repo")

import numpy as np
import ml_dtypes
from operator import add as _op_add

from concourse import bass, bacc, tile, mybir
from concourse import dve_ops as _dve_ops
from concourse.dve_spec import C0, C1, Spec, Src0, Src1, Zero, minn
from concourse.dve_spec import lower as _dve_lower
from concourse.dve_uop import DveOpSpec
from concourse.bass_utils import run_bass_kernel_spmd

dt = mybir.dt
F32, BF16, FP8 = dt.float32, dt.bfloat16, dt.float8e4
AF = mybir.ActivationFunctionType
ALU = mybir.AluOpType

B, N, D, K = 32, 2048, 512, 256
NCORES = 8
BLOC = B // NCORES           # 4 batches per core
SUP = 512                    # scores-matmul tile (PSUM-bank limited)
WSUP = 1024                  # vIp supertile / ha ACT width
DC = D // 128                # 4 d chunks
KC = K // 128                # 2 k chunks
NEG = 0.01


def _ref_invlrelu_mul_reduce(in0, in1, s0, s1, imm2):
    x = in0.astype(np.float32)
    b = ((np.minimum(x, x * s0) * s1) * in1).astype(np.float32)
    return b, b.reshape(b.shape[0], -1).sum(axis=-1, keepdims=True)


def _register_invlrelu_op():
    """out = (min(in0, in0*C0) * C1) * in1; accum_out = sum(out).

    With C0=100, C1=1/8 and in0 = 8*prelu(g, 0.01) this recomputes
    g * e inline and row-accumulates it: the whole attention tail."""
    name = "INV_LRELU_MUL_REDUCE_ANT"
    for op in _dve_ops.OPS:
        if op.name == name:
            return op
    spec = Spec(
        body=(minn(Src0, Src0 * C0) * C1) * Src1,
        accum=_op_add,
        accum_init=Zero,
        reference=_ref_invlrelu_mul_reduce,
    )
    row = _dve_ops._CUSTOM_DVE_ROW_BASE + len(_dve_ops.OPS)
    assert row < 0x20
    op = _dve_ops.DveOp(name, spec, subdim=False, uops_sha={})
    # self-pin the lowering sha (the pin guards cross-version drift; we
    # lower and pin in the same process)
    for ver in ("v3", "v4"):
        try:
            r = DveOpSpec(
                name=name, opcode=row, uops=_dve_lower(spec, ver=ver), rd1_en=True
            )
            op.uops_sha[ver] = r.sha(ver)
        except Exception:
            pass
    _dve_ops.OPS.append(op)
    _dve_ops.CUSTOM_DVE_SPECS[name] = spec
    _dve_ops._SUB_OPCODE_FOR_NAME[name] = row
    return op


INVLRELU_OP = _register_invlrelu_op()


def build_nc():
    nc = bacc.Bacc("TRN2", target_bir_lowering=False, debug=False)

    vit_d = nc.dram_tensor("vit", [BLOC, 128, 2, 2, N], FP8, kind="ExternalInput")
    f8pk_d = nc.dram_tensor("f8pk", [128, 1280], FP8, kind="ExternalInput")
    pk32_d = nc.dram_tensor("pk32", [128, 137], F32, kind="ExternalInput")
    out = nc.dram_tensor("out", [BLOC, K], F32, kind="ExternalOutput")

    DEBUG = bool(int(os.environ.get("KERNEL_DEBUG", "0")))
    DBG_B = int(os.environ.get("KERNEL_DEBUG_B", "0"))
    if DEBUG:
        d_erow = nc.dram_tensor("d_erow", [1, N], FP8, kind="ExternalOutput")
        d_z = nc.dram_tensor("d_z", [1, 1], F32, kind="ExternalOutput")
        d_fin = nc.dram_tensor("d_fin", [1, K], F32, kind="ExternalOutput")

    with tile.TileContext(nc) as tc:
        with (
            tc.tile_pool(name="const", bufs=1) as cpool,
            tc.tile_pool(name="stream", bufs=4) as spool,
            tc.tile_pool(name="work", bufs=2) as wpool,
            tc.tile_pool(name="pmm", bufs=2, space=bass.MemorySpace.PSUM) as pmm,
            tc.tile_pool(name="psm", bufs=1, space=bass.MemorySpace.PSUM) as psm,
        ):
            f8pk_sb = cpool.tile([128, 1280], FP8, tag="f8pk")
            pk32_sb = cpool.tile([128, 137], F32, tag="pk32")

            vit_tiles = [
                spool.tile([128, 2, 2, N], FP8, tag="vit", name=f"vit{b}")
                for b in range(BLOC)
            ]

            # The sync-engine HWDGE queue measured ~28 GB/s (its sequencer is
            # saturated with semaphore traffic) while the ACT HWDGE and
            # gpsimd SWDGE queues sustain ~145 GB/s each -- so ALL bulk vit
            # streams go on those two; sync only carries the small weights.
            nc.sync.dma_start(out=f8pk_sb[:], in_=f8pk_d[:])
            nc.sync.dma_start(out=pk32_sb[:], in_=pk32_d[:])

            # the ACT HWDGE queue is the only consistently fast one
            # (~160 GB/s); keep every vit tile on it in consumption order
            nc.scalar.dma_start(
                out=vit_tiles[0][:, :, :, 0:512], in_=vit_d[0][:, :, :, 0:512]
            )
            nc.scalar.dma_start(
                out=vit_tiles[0][:, :, :, 512:1024], in_=vit_d[0][:, :, :, 512:1024]
            )
            nc.scalar.dma_start(
                out=vit_tiles[0][:, :, :, 1024:N], in_=vit_d[0][:, :, :, 1024:N]
            )
            nc.scalar.dma_start(out=vit_tiles[1][:], in_=vit_d[1])
            nc.scalar.dma_start(out=vit_tiles[2][:], in_=vit_d[2])
            nc.scalar.dma_start(out=vit_tiles[3][:], in_=vit_d[3])

            wi8_sb = f8pk_sb[:, 0:1024].rearrange("p (c i k) -> p c i k", c=2, i=2)
            # wp replicated across all 128 lhsT columns: the scores matmul
            # then writes scores to EVERY partition -- it is the broadcast
            wp8r_sb = f8pk_sb[:, 1024:1280].rearrange("p (i j) -> p i j", i=2)
            vqpt_sb = pk32_sb[:, 0:8].rearrange("p (c b) -> p c b", c=KC)
            idf_sb = pk32_sb[:, 9:137]

            out_sb = cpool.tile([1, BLOC, K], F32, tag="outb")
            has = [None] * BLOC
            accs = [None] * BLOC
            invzs = [None] * BLOC

            def phase_scores(b):
                vit = vit_tiles[b]
                # ha stays alive until the attention reduce of batch b
                ha = wpool.tile([128, KC, N], FP8, tag="ha")
                has[b] = ha
                e_b = wpool.tile([128, N], FP8, tag="eb")
                zq = wpool.tile([128, 1], F32, tag="zq")
                # all four score quarters accumulate into one 4-bank PSUM
                # tile; a single wide exp then drains it
                scp = psm.tile([128, N], F32, tag="scp", name=f"scp{b}")
                # batch 0 uses two narrow leading supertiles so the first
                # ACT fires as soon as the first quarter of vit0 lands
                widths = (SUP, SUP, WSUP) if b == 0 else (WSUP, WSUP)
                n0 = 0
                for w in widths:
                    for kc in range(KC):
                        vp = pmm.tile([128, w], F32, tag="vp", name=f"vp{b}_{n0}_{kc}")
                        for h in range(w // SUP):
                            for cc in range(2):
                                nc.tensor.matmul(
                                    vp[:, h * SUP : (h + 1) * SUP],
                                    wi8_sb[:, cc, :, kc * 128 : (kc + 1) * 128],
                                    vit[:, cc, :, n0 + h * SUP : n0 + (h + 1) * SUP],
                                    perf_mode=mybir.MatmulPerfMode.DoubleRow,
                                    start=(cc == 0),
                                    stop=(cc == 1),
                                )
                        # ha8 = 8*prelu(g): vp = 16*vIp, scale 0.5 -> 8*vIp,
                        # bias = 8*vQp (host). Prelu shares the ACT table
                        # with Exp: zero reloads.
                        nc.scalar.activation(
                            ha[:, kc, n0 : n0 + w], vp[:], AF.Prelu,
                            bias=vqpt_sb[:, kc, b : b + 1], scale=0.5, alpha=NEG,
                        )
                    for h in range(w // SUP):
                        nc.tensor.matmul(
                            scp[:, n0 + h * SUP : n0 + (h + 1) * SUP], wp8r_sb[:],
                            ha[:, :, n0 + h * SUP : n0 + (h + 1) * SUP],
                            perf_mode=mybir.MatmulPerfMode.DoubleRow,
                            start=True, stop=True,
                        )
                    n0 += w
                invz = wpool.tile([1, 1], F32, tag="invz", name=f"invz{b}")
                invzs[b] = invz
                acc = wpool.tile([128, KC], F32, tag="acc", name=f"acc{b}")
                accs[b] = acc
                scr = wpool.tile([128, N], FP8, tag="scr")
                if b < BLOC - 1:
                    # one wide partition-parallel exp: e_b for all partitions,
                    # Z replicated into every partition of zq via accum
                    nc.scalar.activation(
                        e_b[:], scp[:], AF.Exp, scale=1.0 / 8, accum_out=zq[:],
                    )
                    nc.vector.reciprocal(invz[:], zq[0:1, :])
                    # att^T[k] = sum_n e[n] * g[k,n]: one fused pass per kc
                    for kc in range(KC):
                        nc.vector._custom_dve(
                            INVLRELU_OP,
                            out=scr[:],
                            in0=ha[:, kc, :],
                            in1=e_b[:],
                            s0=100.0,
                            s1=1.0 / 8,
                            accum_out=acc[:, kc : kc + 1],
                        )
                else:
                    # last batch: halve the exp->reduce chain so the kernel
                    # drain is ~2 us shorter
                    zq2 = wpool.tile([128, 2], F32, tag="zq2")
                    acch = wpool.tile([128, KC, 2], F32, tag="acch")
                    HN = N // 2
                    for hh in range(2):
                        nc.scalar.activation(
                            e_b[:, hh * HN : (hh + 1) * HN],
                            scp[:, hh * HN : (hh + 1) * HN],
                            AF.Exp, scale=1.0 / 8,
                            accum_out=zq2[:, hh : hh + 1],
                        )
                        for kc in range(KC):
                            nc.vector._custom_dve(
                                INVLRELU_OP,
                                out=scr[:, 0:HN],
                                in0=ha[:, kc, hh * HN : (hh + 1) * HN],
                                in1=e_b[:, hh * HN : (hh + 1) * HN],
                                s0=100.0,
                                s1=1.0 / 8,
                                accum_out=acch[:, kc, hh : hh + 1],
                            )
                    z = wpool.tile([1, 1], F32, tag="z")
                    nc.vector.tensor_tensor(
                        z[:], zq2[0:1, 0:1], zq2[0:1, 1:2], ALU.add
                    )
                    nc.vector.reciprocal(invz[:], z[:])
                    nc.vector.tensor_tensor(
                        acc[:], acch[:, :, 0], acch[:, :, 1], ALU.add
                    )
                if DEBUG and b == DBG_B:
                    nc.sync.dma_start(out=d_erow[:], in_=e_b[0:1, :])
                    nc.sync.dma_start(out=d_z[:], in_=zq[0:1, :])

            def phase_attn(b):
                # transpose att^T back to a [1, K] row and scale by 1/Z
                acc, invz = accs[b], invzs[b]
                outp = pmm.tile([1, K], F32, tag="vp", name=f"outp{b}")
                for kc in range(KC):
                    nc.tensor.transpose(
                        outp[0:1, kc * 128 : (kc + 1) * 128],
                        acc[:, kc : kc + 1],
                        idf_sb[:],
                    )
                nc.vector.tensor_scalar(
                    out_sb[:, b, :], outp[:], invz[:], None, ALU.mult
                )
                if DEBUG and b == DBG_B:
                    nc.sync.dma_start(out=d_fin[:], in_=out_sb[0:1, b, :])

            # attention-tail PE work (2 tiny transposes) trails by one
            # phase so the DVE reduce has a full scores phase to finish
            for b in range(BLOC + 1):
                if b < BLOC:
                    phase_scores(b)
                if b >= 1:
                    phase_attn(b - 1)

            nc.sync.dma_start(out=out[:, :], in_=out_sb[0:1, :, :])

    nc.compile()
    return nc


_NC = None


def _get_nc():
    global _NC
    if _NC is None:
        _NC = build_nc()
    return _NC


def kernel(vI, vQ, Wi, Wq, bq, Wp, bp, **_unused):
    vI = np.asarray(vI, dtype=np.float32)
    vQ = np.asarray(vQ, dtype=np.float32)
    Wi = np.asarray(Wi, dtype=np.float32)
    Wq = np.asarray(Wq, dtype=np.float32)
    bq = np.asarray(bq, dtype=np.float32)
    Wp = np.asarray(Wp, dtype=np.float32)
    # bp shifts every score equally -> cancels in softmax; ignored.

    f8 = ml_dtypes.float8_e4m3
    vi8 = vI.astype(f8)
    # DoubleRow layout: d = cc*256 + i*128 + p  ->  [B, p, cc, i, N]
    viT = np.ascontiguousarray(
        vi8.transpose(0, 2, 1).reshape(B, 2, 2, 128, N).transpose(0, 3, 1, 2, 4)
    )

    vQp = vQ @ Wq + bq                                           # [B, K] fp32

    wi8_dr = np.ascontiguousarray(
        (Wi * 16.0).reshape(2, 2, 128, K).transpose(2, 0, 1, 3)
    ).reshape(128, 1024)                                          # [128,(cc i K)]
    # ha carries 8x scale; wp stays 1x so scp = 8*scores (exp scale 1/8)
    wp_h = Wp[:, 0].reshape(KC, 128).T                           # [128,KC]
    wp_rep = np.repeat(wp_h[:, :, None], 128, axis=2)            # [128,2,128]
    f8pk = np.concatenate(
        [wi8_dr, wp_rep.reshape(128, 256)], axis=1
    ).astype(f8)                                                  # [128,1280]

    onesc = np.ones((128, 1), np.float32)
    idf = np.eye(128, dtype=np.float32)

    def pk32_for(core):
        vqpc = 8.0 * vQp[core * BLOC : (core + 1) * BLOC]         # [BLOC, K]
        vqpt = vqpc.T.reshape(KC, 128, BLOC).transpose(1, 0, 2)   # [128,KC,BLOC]
        return np.ascontiguousarray(
            np.concatenate([vqpt.reshape(128, KC * BLOC), onesc, idf], axis=1)
        ).astype(np.float32)                                      # [128,137]

    in_maps = []
    for c in range(NCORES):
        in_maps.append(
            {
                "vit": viT[c * BLOC : (c + 1) * BLOC],
                "f8pk": f8pk,
                "pk32": pk32_for(c),
            }
        )

    nc = _get_nc()
    res = run_bass_kernel_spmd(
        nc, in_maps, list(range(NCORES)),
        trace=bool(int(os.environ.get("KERNEL_TRACE", "0"))),
        tmpdir=globals().get("TRACE_TMPDIR"),
    )
    kernel.last_results = res
    return np.concatenate([res.results[c]["out"] for c in range(NCORES)], axis=0)
